# revision 1
# baseline (speedup 1.0000x reference)
"""GCNII layer on 8 TRN2 NeuronCores (Bass/Tile).

Strategy: partition nodes (and their incoming edges, bucketed by dst) across
the 8 cores; replicate the feature table (bf16) in every core's DRAM.  Per
core, nodes are load-balanced into 98 chunks of 128 output slots (serpentine
by in-degree).  Edges are grouped (chunk, src-subrange) with the feature
table split into 4 sub-tables of <32768 rows so dma_gather's int16 indices
reach every row.  Gathers run one dma_gather per (7-chunk group, subrange).
Per 128-edge tile, a one-hot selection matrix (iota == slot) scaled by
rsqrt(deg[src]) is built in one fused DVE op, and TensorE matmuls accumulate
the degree-normalized neighbor sum in fp32 PSUM.  The epilogue applies
rsqrt(deg[dst]), the alpha initial-residual blend, the identity-mapped W
matmul (via a PE transpose) and the fused ReLU.  Host-side work is integer
bucketing/layout only; all float math runs on device.
"""

import sys

if "/opt/trn_rl_repo" not in sys.path:
    sys.path.insert(0, "/opt/trn_rl_repo")

from contextlib import ExitStack

import ml_dtypes
import numpy as np

N, E, D, NC = 100000, 1600000, 128, 8
NPC = N // NC            # nodes per core: 12500
CHUNKS = 98              # chunks of 128 output slots per core
SLOTS = CHUNKS * 128     # padded node slots per core: 12544
ALPHA, BETA = 0.1, 0.5
NSUB = 4                 # feature-table subranges (int16 index limit)
SR = 25000               # rows per subrange

F32 = np.float32
BF16 = ml_dtypes.bfloat16


def _wrap_idx(seq):
    """dma_gather index layout: i -> [i % 16, i // 16], replicated to 128
    partitions (one copy per Q7 core)."""
    blk = seq.reshape(-1, 16).T
    return np.tile(blk, (8, 1))


def _host_prep(features, initial_features, W, src, dst):
    """Integer-only bucketing/layout prep -> per-core device arrays."""
    src = np.ascontiguousarray(src).astype(np.int64, copy=False)
    dst = np.ascontiguousarray(dst).astype(np.int64, copy=False)
    deg = np.bincount(dst, minlength=N)
    degc = np.maximum(deg, 1).astype(F32)
    core_of = dst // NPC
    cores_tmp = []
    max_sub_load = 0
    for c in range(NC):
        em = core_of == c
        e_src = src[em]
        e_loc = dst[em] - c * NPC
        ndeg = deg[c * NPC:(c + 1) * NPC]
        order = np.argsort(-ndeg, kind="stable")
        order_p = np.concatenate([order, np.full(SLOTS - NPC, -1, np.int64)])
        arr = order_p.reshape(128, CHUNKS)
        arr[1::2] = arr[1::2, ::-1]          # serpentine -> balanced chunk loads
        nodelist = arr.T.copy()              # [98,128] local node id or -1
        chunk_of = np.empty(NPC, np.int64)
        slot_of = np.empty(NPC, np.int64)
        ch = np.repeat(np.arange(CHUNKS), 128).reshape(CHUNKS, 128)
        sl = np.tile(np.arange(128), (CHUNKS, 1))
        v = nodelist >= 0
        chunk_of[nodelist[v]] = ch[v]
        slot_of[nodelist[v]] = sl[v]
        e_chunk = chunk_of[e_loc]
        e_slot = slot_of[e_loc]
        o = np.lexsort((e_src, e_chunk))     # chunk-major, src-sorted within
        e_src, e_slot, e_chunk = e_src[o], e_slot[o], e_chunk[o]
        e_sub = e_src // SR
        cnt = np.bincount(e_chunk * NSUB + e_sub, minlength=CHUNKS * NSUB)
        max_sub_load = max(max_sub_load, int(cnt.max()))
        cores_tmp.append((e_src, e_slot, e_chunk, e_sub, cnt, nodelist))
    TR = (max_sub_load + 127) // 128         # tiles per (chunk, subrange)
    TT = NSUB * TR                           # tiles per chunk
    cap = TR * 128
    per_core = []
    for c in range(NC):
        e_src, e_slot, e_chunk, e_sub, cnt, nodelist = cores_tmp[c]
        # [98, NSUB, TR*128] per-(chunk,subrange) padded segments
        idx_arr = np.zeros((CHUNKS, NSUB, cap), np.int16)
        rel_arr = np.full((CHUNKS, NSUB, cap), -1.0, F32)
        dsg_arr = np.ones((CHUNKS, NSUB, cap), F32)
        cnt2 = cnt.reshape(CHUNKS, NSUB).astype(np.int32)
        starts = np.zeros(CHUNKS * NSUB, np.int64)
        starts[1:] = np.cumsum(cnt)[:-1]
        pos = np.arange(len(e_src)) - starts[e_chunk * NSUB + e_sub]
        idx_arr[e_chunk, e_sub, pos] = (e_src - e_sub * SR).astype(np.int16)
        rel_arr[e_chunk, e_sub, pos] = e_slot
        dsg_arr[e_chunk, e_sub, pos] = degc[e_src]
        # device order: (chunk c, subrange r, tile t, part p)
        idx_dev = _wrap_idx(idx_arr.reshape(-1)).astype(np.int16)   # [128, CHUNKS*TT*8]

        def dev(a):   # [98, NSUB, cap] -> [128, CHUNKS*NSUB*TR] in device order
            return np.ascontiguousarray(a.reshape(CHUNKS * TT, 128).T)

        glob = np.where(nodelist >= 0, nodelist + c * NPC, -1)
        init_perm = np.zeros((SLOTS, D), F32)
        gv = glob.reshape(-1)
        init_perm[gv >= 0] = initial_features[gv[gv >= 0]]
        dcd = np.ones((CHUNKS, 128), F32)
        dcd[glob >= 0] = degc[glob[glob >= 0]]
        ncalls = CHUNKS * NSUB
        qcols = (ncalls + 127) // 128
        cnt_dev = np.zeros((128, qcols), np.int32)
        flat = cnt2.reshape(-1)
        kk = np.arange(ncalls)
        cnt_dev[kk % 128, kk // 128] = flat
        per_core.append(
            dict(
                ecnt=cnt_dev,
                eidx=np.ascontiguousarray(idx_dev),
                erel=dev(rel_arr),
                edsg=dev(dsg_arr),
                dcd=np.ascontiguousarray(dcd.T),
                initp=init_perm,
                glob=glob,
            )
        )
    return per_core, TR


_BUILD_CACHE = {}


def _build(TR, n_rows=N, chunks=CHUNKS, nsub=NSUB, sr=SR):
    key = (TR, n_rows, chunks, nsub, sr)
    if key in _BUILD_CACHE:
        return _BUILD_CACHE[key]
    import concourse.bacc as bacc
    import concourse.bass as bass  # noqa: F401
    import concourse.mybir as mybir
    import concourse.tile as tile

    f32 = mybir.dt.float32
    bf16 = mybir.dt.bfloat16
    i16 = mybir.dt.int16
    Alu = mybir.AluOpType
    Act = mybir.ActivationFunctionType

    TT = nsub * TR
    SLOTS_ = chunks * 128
    COLS = chunks * TT               # total edge-tile columns
    IDXC = COLS * 8                  # idx cols (int16, 16-wrap => /16*128)

    nc = bacc.Bacc("TRN2", target_bir_lowering=False, num_swdge_queues=4)
    feats = nc.dram_tensor("feats", [n_rows, D], bf16, kind="ExternalInput")
    wt = nc.dram_tensor("wt", [D, D], f32, kind="ExternalInput")
    iota = nc.dram_tensor("iota", [128, 128], bf16, kind="ExternalInput")
    ident = nc.dram_tensor("ident", [128, 128], f32, kind="ExternalInput")
    eidx = nc.dram_tensor("eidx", [128, IDXC], i16, kind="ExternalInput")
    erel = nc.dram_tensor("erel", [128, COLS], f32, kind="ExternalInput")
    edsg = nc.dram_tensor("edsg", [128, COLS], f32, kind="ExternalInput")
    dcd = nc.dram_tensor("dcd", [128, chunks], f32, kind="ExternalInput")
    initp = nc.dram_tensor("initp", [SLOTS_, D], f32, kind="ExternalInput")
    out = nc.dram_tensor("out", [SLOTS_, D], f32, kind="ExternalOutput")

    with tile.TileContext(nc) as tc, ExitStack() as ctx:
        const = ctx.enter_context(tc.tile_pool(name="const", bufs=1))
        gpool = ctx.enter_context(tc.tile_pool(name="g", bufs=12))
        ohpool = ctx.enter_context(tc.tile_pool(name="oh", bufs=144))
        epool = ctx.enter_context(tc.tile_pool(name="ep", bufs=4))
        ipool = ctx.enter_context(tc.tile_pool(name="init", bufs=3))
        opool = ctx.enter_context(tc.tile_pool(name="ob", bufs=3))
        ps_agg = ctx.enter_context(tc.tile_pool(name="psagg", bufs=4, space="PSUM"))
        ps_tr = ctx.enter_context(tc.tile_pool(name="pstr", bufs=2, space="PSUM"))
        ps_mm = ctx.enter_context(tc.tile_pool(name="psmm", bufs=2, space="PSUM"))

        iota_sb = const.tile([128, 128], bf16)
        nc.sync.dma_start(out=iota_sb[:], in_=iota[:])
        wt_sb = const.tile([128, 128], f32)
        nc.sync.dma_start(out=wt_sb[:], in_=wt[:])
        id_sb = const.tile([128, 128], f32)
        nc.sync.dma_start(out=id_sb[:], in_=ident[:])
        idx_sb = const.tile([128, IDXC], i16)
        nc.sync.dma_start(out=idx_sb[:], in_=eidx[:])
        rel_sb = const.tile([128, COLS], f32)
        nc.sync.dma_start(out=rel_sb[:], in_=erel[:])
        dsg_sb = const.tile([128, COLS], f32)
        nc.sync.dma_start(out=dsg_sb[:], in_=edsg[:])
        dcd_sb = const.tile([128, chunks], f32)
        nc.sync.dma_start(out=dcd_sb[:], in_=dcd[:])

        nsrcf_sb = const.tile([128, COLS], f32)
        nc.scalar.activation(nsrcf_sb[:], dsg_sb[:], Act.Sqrt)
        nc.vector.reciprocal(nsrcf_sb[:], nsrcf_sb[:])
        ndst_sb = const.tile([128, chunks], f32)
        nc.scalar.activation(ndst_sb[:], dcd_sb[:], Act.Sqrt)
        nc.vector.reciprocal(ndst_sb[:], ndst_sb[:])
        nc.vector.tensor_scalar(ndst_sb[:], ndst_sb[:], 1.0 - ALPHA, None, Alu.mult)

        ni = TR * 128
        for c in range(chunks):
            buf = gpool.tile([128, TT * 128], bf16)
            for r in range(nsub):
                lo = r * sr
                hi = min(n_rows, (r + 1) * sr)
                callbase = (c * TT + r * TR) * 8
                nc.gpsimd.dma_gather(
                    out_ap=buf[:, r * TR * 128:(r + 1) * TR * 128]
                    .rearrange("p (t d) -> p t d", t=TR),
                    in_ap=feats[lo:hi, :],
                    idxs_ap=idx_sb[:, callbase:callbase + TR * 8],
                    num_idxs=ni,
                    num_idxs_reg=ni,
                    elem_size=D,
                    single_packet=False,
                    queue_num=(c * nsub + r) % 4,
                )
            if True:
                psum = ps_agg.tile([128, 128], f32, space="PSUM")
                for k in range(TT):
                    col = c * TT + k
                    oh = ohpool.tile([128, 128], bf16)
                    nc.vector.tensor_scalar(
                        oh[:],
                        iota_sb[:],
                        rel_sb[:, col:col + 1],
                        nsrcf_sb[:, col:col + 1],
                        Alu.is_equal,
                        Alu.mult,
                    )
                    nc.tensor.matmul(
                        psum[:],
                        lhsT=oh[:],
                        rhs=buf[:, k * 128:(k + 1) * 128],
                        start=(k == 0),
                        stop=(k == TT - 1),
                    )
                itile = ipool.tile([128, 128], f32)
                nc.sync.dma_start(out=itile[:], in_=initp[c * 128:(c + 1) * 128, :])
                h2 = epool.tile([128, 128], f32, tag="h2")
                nc.scalar.activation(h2[:], psum[:], Act.Copy,
                                     scale=ndst_sb[:, c:c + 1])
                isc = epool.tile([128, 128], f32, tag="isc")
                nc.scalar.activation(isc[:], itile[:], Act.Copy, scale=ALPHA)
                # (h2 + isc).T accumulated in PSUM via two transpose-matmuls
                ptr = ps_tr.tile([128, 128], f32, space="PSUM")
                nc.tensor.matmul(ptr[:], lhsT=h2[:], rhs=id_sb[:],
                                 start=True, stop=False)
                nc.tensor.matmul(ptr[:], lhsT=isc[:], rhs=id_sb[:],
                                 start=False, stop=True)
                h3t = epool.tile([128, 128], f32, tag="h3t")
                nc.scalar.activation(h3t[:], ptr[:], Act.Copy)
                # h3 @ W.T + h3 accumulated in PSUM
                pmm = ps_mm.tile([128, 128], f32, space="PSUM")
                nc.tensor.matmul(
                    pmm[:], lhsT=h3t[:], rhs=wt_sb[:], start=True, stop=False
                )
                nc.tensor.matmul(
                    pmm[:], lhsT=h3t[:], rhs=id_sb[:], start=False, stop=True
                )
                ob = opool.tile([128, 128], f32)
                nc.scalar.activation(ob[:], pmm[:], Act.Relu, scale=BETA)
                nc.sync.dma_start(out=out[c * 128:(c + 1) * 128, :], in_=ob[:])

    nc.compile()
    _BUILD_CACHE[key] = nc
    return nc


def _install_ntff_shim():
    """antenv.axon_hooks is absent in this image; shim it and wire the real
    NTFF profiling hook via ctypes so trace=True works under axon."""
    import contextlib
    import ctypes
    import types

    try:
        from antenv import axon_hooks  # noqa: F401
        return
    except ImportError:
        pass
    import antenv

    mod = types.ModuleType("antenv.axon_hooks")
    _hook = [None]
    mod.set_axon_ntff_profile_hook = lambda h: _hook.__setitem__(0, h)
    mod.get_axon_ntff_profile_hook = lambda: _hook[0]
    sys.modules["antenv.axon_hooks"] = mod
    antenv.axon_hooks = mod
    try:
        lib = ctypes.CDLL("/opt/axon/libaxon_pjrt.so")
    except OSError:
        return
    if not hasattr(lib, "axon_start_nrt_profile"):
        return
    lib.axon_start_nrt_profile.argtypes = [
        ctypes.POINTER(ctypes.c_int64),
        ctypes.c_size_t,
    ]
    lib.axon_start_nrt_profile.restype = ctypes.c_int64
    lib.axon_stop_nrt_profile.argtypes = [ctypes.c_char_p]
    lib.axon_stop_nrt_profile.restype = ctypes.c_int64

    @contextlib.contextmanager
    def _hook_cm(output_dir, device_ids):
        import jax

        jax.devices()
        if device_ids:
            ids = (ctypes.c_int64 * len(device_ids))(*device_ids)
            rc = lib.axon_start_nrt_profile(ids, len(device_ids))
        else:
            rc = lib.axon_start_nrt_profile(None, 0)
        if rc != 0:
            raise RuntimeError(f"axon_start_nrt_profile rc={rc}")
        try:
            yield
        finally:
            rc = lib.axon_stop_nrt_profile(output_dir.encode())
            if rc != 0:
                print(f"WARNING: axon_stop_nrt_profile rc={rc}", flush=True)

    mod.set_axon_ntff_profile_hook(_hook_cm)


def _run(inputs, trace=False, trace_cores=None):
    from concourse import bass_utils

    if trace:
        _install_ntff_shim()
    features = np.ascontiguousarray(np.asarray(inputs["features"], dtype=F32))
    initial_features = np.ascontiguousarray(
        np.asarray(inputs["initial_features"], dtype=F32)
    )
    W = np.asarray(inputs["W"], dtype=F32)
    src = np.asarray(inputs["src"])
    dst = np.asarray(inputs["dst"])
    per_core, TR = _host_prep(features, initial_features, W, src, dst)
    nc = _build(TR)
    feats_bf = np.ascontiguousarray(features.astype(BF16))
    wt_np = np.ascontiguousarray(W.T)
    iota_np = np.ascontiguousarray(
        np.tile(np.arange(128, dtype=F32), (128, 1)).astype(BF16)
    )
    ident_np = np.eye(128, dtype=F32)
    in_maps = []
    for c in range(NC):
        pc = per_core[c]
        in_maps.append(
            dict(
                feats=feats_bf,
                wt=wt_np,
                iota=iota_np,
                ident=ident_np,
                eidx=pc["eidx"],
                erel=pc["erel"],
                edsg=pc["edsg"],
                dcd=pc["dcd"],
                initp=pc["initp"],
            )
        )
    res = bass_utils.run_bass_kernel_spmd(
        nc,
        in_maps,
        core_ids=list(range(NC)),
        trace=trace,
        trace_cores=trace_cores,
    )
    result = np.empty((N, D), F32)
    for c in range(NC):
        glob = per_core[c]["glob"].reshape(-1)
        oc = res.results[c]["out"]
        m = glob >= 0
        result[glob[m]] = oc[m]
    return result, res


def kernel(**inputs):
    return _run(inputs, trace=False)[0]



# revision 3
# speedup vs baseline: 1.2218x; 1.2218x over previous
"""GCNII layer on 8 TRN2 NeuronCores (Bass/Tile).

Strategy: partition nodes (and their incoming edges, bucketed by dst) across
the 8 cores; replicate the feature table (bf16) in every core's DRAM.  Per
core, nodes are greedily packed into chunks of 128 output slots balancing the
per-(chunk, src-subrange) edge counts so each (chunk,sub) fits TR 128-edge
tiles.  The feature table is split into 4 sub-tables of <32768 rows so
dma_gather's int16 indices reach every row; gathers are batched one call per
(8-chunk group, subrange) to amortize SWDGE descriptor-generation.  Per
128-edge tile a one-hot selection matrix (iota == slot) scaled by
rsqrt(deg[src]) is built — load-balanced across the Vector, GpSimd and Scalar
engines (Scalar uses an |x| / ReLU identity to express the same one-hot) —
and TensorE matmuls accumulate the degree-normalized neighbor sum in fp32
PSUM.  The epilogue applies rsqrt(deg[dst]), the alpha initial-residual blend
(folded into a 0.1*I transpose-matmul), the identity-mapped W matmul and the
fused ReLU.  Host-side work is integer bucketing/layout only; all float math
runs on device.
"""

import sys

if "/opt/trn_rl_repo" not in sys.path:
    sys.path.insert(0, "/opt/trn_rl_repo")

from contextlib import ExitStack

import ml_dtypes
import numpy as np

N, E, D, NC = 100000, 1600000, 128, 8
NPC = N // NC            # nodes per core: 12500
ALPHA, BETA = 0.1, 0.5
NSUB = 4                 # feature-table subranges (int16 index limit)
SR = 25000               # rows per subrange

F32 = np.float32
BF16 = ml_dtypes.bfloat16

# one-hot build engine split per chunk's TT tiles (k % TT):
#   indices in _DVE_K -> vector, _GP_K -> gpsimd, rest -> scalar(2-op)
def _engine_of(k, TT):
    m = k % 16
    if m < 10:
        return "dve"
    if m < 14:
        return "gp"
    return "act"


def _wrap_idx(seq):
    """dma_gather index layout: i -> [i % 16, i // 16], replicated to 128
    partitions (one copy per Q7 core)."""
    blk = seq.reshape(-1, 16).T
    return np.tile(blk, (8, 1))


def _balance_nodes(deg_sub, chunks):
    """Greedy 4-dim balancing: assign nodes (rows of deg_sub [n,4]) to
    `chunks` bins (<=128 nodes each) minimizing max per-(bin,sub) load.
    Returns chunk_of, slot_of arrays."""
    n = deg_sub.shape[0]
    order = np.argsort(-deg_sub.sum(1), kind="stable")
    loads = np.zeros((chunks, NSUB), np.int64)
    counts = np.zeros(chunks, np.int64)
    chunk_of = np.empty(n, np.int64)
    slot_of = np.empty(n, np.int64)
    for i in order:
        score = np.max(loads + deg_sub[i], axis=1)
        score[counts >= 128] = 1 << 60
        c = int(np.argmin(score))
        chunk_of[i] = c
        slot_of[i] = counts[c]
        counts[c] += 1
        loads[c] += deg_sub[i]
    return chunk_of, slot_of, loads


def _host_prep(features, initial_features, W, src, dst):
    """Integer-only bucketing/layout prep -> per-core device arrays."""
    src = np.ascontiguousarray(src).astype(np.int64, copy=False)
    dst = np.ascontiguousarray(dst).astype(np.int64, copy=False)
    deg = np.bincount(dst, minlength=N)
    degc = np.maximum(deg, 1).astype(F32)
    core_of = dst // NPC

    CHUNKS = 104
    cores_tmp = []
    max_load = 0
    for c in range(NC):
        em = core_of == c
        e_src = src[em]
        e_loc = dst[em] - c * NPC
        e_sub = e_src // SR
        deg_sub = np.zeros((NPC, NSUB), np.int64)
        np.add.at(deg_sub, (e_loc, e_sub), 1)
        chunk_of, slot_of, loads = _balance_nodes(deg_sub, CHUNKS)
        max_load = max(max_load, int(loads.max()))
        cores_tmp.append((e_src, e_loc, e_sub, chunk_of, slot_of))
    TR = (max_load + 127) // 128         # tiles per (chunk, subrange)
    TT = NSUB * TR                       # tiles per chunk
    cap = TR * 128
    SLOTS = CHUNKS * 128
    G = 8 if CHUNKS % 8 == 0 else 7      # chunks per gather group
    per_core = []
    for c in range(NC):
        e_src, e_loc, e_sub, chunk_of, slot_of = cores_tmp[c]
        e_chunk = chunk_of[e_loc]
        e_slot = slot_of[e_loc]
        o = np.lexsort((e_src, e_sub, e_chunk))
        e_src, e_slot, e_chunk, e_sub = e_src[o], e_slot[o], e_chunk[o], e_sub[o]
        cnt = np.bincount(e_chunk * NSUB + e_sub, minlength=CHUNKS * NSUB)
        # [CHUNKS, NSUB, cap] per-(chunk,subrange) padded segments
        idx_arr = np.zeros((CHUNKS, NSUB, cap), np.int16)
        rel_arr = np.full((CHUNKS, NSUB, cap), -1.0, F32)
        dsg_arr = np.ones((CHUNKS, NSUB, cap), F32)
        starts = np.zeros(CHUNKS * NSUB, np.int64)
        starts[1:] = np.cumsum(cnt)[:-1]
        pos = np.arange(len(e_src)) - starts[e_chunk * NSUB + e_sub]
        idx_arr[e_chunk, e_sub, pos] = (e_src - e_sub * SR).astype(np.int16)
        rel_arr[e_chunk, e_sub, pos] = e_slot
        dsg_arr[e_chunk, e_sub, pos] = degc[e_src]
        # gather-call order: (group g, sub r, chunk-local, tile t, part p)
        NG = CHUNKS // G
        idx_g = idx_arr.reshape(NG, G, NSUB, cap)
        idx_flat = idx_g.transpose(0, 2, 1, 3).reshape(-1)
        idx_dev = _wrap_idx(idx_flat).astype(np.int16)   # [128, COLS*8]

        def dev(a):   # [CHUNKS, NSUB, cap] -> [128, COLS] table, col=c*TT+k
            return np.ascontiguousarray(a.reshape(CHUNKS * TT, 128).T)

        # node layout tables
        nodelist = np.full((CHUNKS, 128), -1, np.int64)
        nodelist[chunk_of, slot_of] = np.arange(NPC)
        glob = np.where(nodelist >= 0, nodelist + c * NPC, -1)
        init_perm = np.zeros((SLOTS, D), F32)
        gv = glob.reshape(-1)
        init_perm[gv >= 0] = initial_features[gv[gv >= 0]]
        dcd = np.ones((CHUNKS, 128), F32)
        dcd[glob >= 0] = degc[glob[glob >= 0]]
        per_core.append(
            dict(
                eidx=np.ascontiguousarray(idx_dev),
                erel=np.ascontiguousarray(dev(rel_arr)),
                negrel=np.ascontiguousarray(dev(-rel_arr)),
                edsg=dev(dsg_arr),
                dcd=np.ascontiguousarray(dcd.T),
                initp=init_perm,
                glob=glob,
            )
        )
    return per_core, TR, CHUNKS, G


_BUILD_CACHE = {}


def _build(TR, chunks, G, n_rows=N, nsub=NSUB, sr=SR):
    key = (TR, chunks, G, n_rows, nsub, sr)
    if key in _BUILD_CACHE:
        return _BUILD_CACHE[key]
    import concourse.bacc as bacc
    import concourse.bass as bass  # noqa: F401
    import concourse.mybir as mybir
    import concourse.tile as tile

    f32 = mybir.dt.float32
    bf16 = mybir.dt.bfloat16
    i16 = mybir.dt.int16
    Alu = mybir.AluOpType
    Act = mybir.ActivationFunctionType

    TT = nsub * TR
    SLOTS_ = chunks * 128
    COLS = chunks * TT               # total edge-tile columns
    IDXC = COLS * 8                  # idx cols (int16, 16-wrap => /16*128)
    NG = chunks // G                 # gather groups
    GTILES = G * TT                  # tiles per group
    NIC = G * TR * 128               # idxs per gather call
    ICC = NIC // 16                  # idx cols per gather call

    nc = bacc.Bacc("TRN2", target_bir_lowering=False, num_swdge_queues=4)
    feats = nc.dram_tensor("feats", [n_rows, D], bf16, kind="ExternalInput")
    wt = nc.dram_tensor("wt", [D, D], f32, kind="ExternalInput")
    iota = nc.dram_tensor("iota", [128, 128], bf16, kind="ExternalInput")
    ident = nc.dram_tensor("ident", [128, 128], f32, kind="ExternalInput")
    ident01 = nc.dram_tensor("ident01", [128, 128], f32, kind="ExternalInput")
    eidx = nc.dram_tensor("eidx", [128, IDXC], i16, kind="ExternalInput")
    erel = nc.dram_tensor("erel", [128, COLS], f32, kind="ExternalInput")
    negrel = nc.dram_tensor("negrel", [128, COLS], f32, kind="ExternalInput")
    edsg = nc.dram_tensor("edsg", [128, COLS], f32, kind="ExternalInput")
    dcd = nc.dram_tensor("dcd", [128, chunks], f32, kind="ExternalInput")
    initp = nc.dram_tensor("initp", [SLOTS_, D], f32, kind="ExternalInput")
    out = nc.dram_tensor("out", [SLOTS_, D], f32, kind="ExternalOutput")

    with tile.TileContext(nc) as tc, ExitStack() as ctx:
        const = ctx.enter_context(tc.tile_pool(name="const", bufs=1))
        gpool = ctx.enter_context(tc.tile_pool(name="g", bufs=2))
        ohpool = ctx.enter_context(tc.tile_pool(name="oh", bufs=96))
        tmppool = ctx.enter_context(tc.tile_pool(name="tmp", bufs=8))
        epool = ctx.enter_context(tc.tile_pool(name="ep", bufs=4))
        ipool = ctx.enter_context(tc.tile_pool(name="init", bufs=3))
        opool = ctx.enter_context(tc.tile_pool(name="ob", bufs=3))
        ps_agg = ctx.enter_context(tc.tile_pool(name="psagg", bufs=4, space="PSUM"))
        ps_tr = ctx.enter_context(tc.tile_pool(name="pstr", bufs=2, space="PSUM"))
        ps_mm = ctx.enter_context(tc.tile_pool(name="psmm", bufs=2, space="PSUM"))

        iota_sb = const.tile([128, 128], bf16)
        nc.sync.dma_start(out=iota_sb[:], in_=iota[:])
        wt_sb = const.tile([128, 128], f32)
        nc.sync.dma_start(out=wt_sb[:], in_=wt[:])
        id_sb = const.tile([128, 128], f32)
        nc.sync.dma_start(out=id_sb[:], in_=ident[:])
        id01_sb = const.tile([128, 128], f32)
        nc.sync.dma_start(out=id01_sb[:], in_=ident01[:])
        idx_sb = const.tile([128, IDXC], i16)
        nc.sync.dma_start(out=idx_sb[:], in_=eidx[:])
        rel_sb = const.tile([128, COLS], f32)
        nc.sync.dma_start(out=rel_sb[:], in_=erel[:])
        nrel_sb = const.tile([128, COLS], f32)
        nc.sync.dma_start(out=nrel_sb[:], in_=negrel[:])
        dsg_sb = const.tile([128, COLS], f32)
        nc.sync.dma_start(out=dsg_sb[:], in_=edsg[:])
        dcd_sb = const.tile([128, chunks], f32)
        nc.sync.dma_start(out=dcd_sb[:], in_=dcd[:])

        # nsrc = rsqrt(deg[src]) tables: f32 (+neg) for ACT, bf16 for DVE/GP
        nsrcf_sb = const.tile([128, COLS], f32)
        nc.scalar.activation(nsrcf_sb[:], dsg_sb[:], Act.Sqrt)
        nc.vector.reciprocal(nsrcf_sb[:], nsrcf_sb[:])
        negns_sb = const.tile([128, COLS], f32)
        nc.vector.tensor_scalar(negns_sb[:], nsrcf_sb[:], -1.0, None, Alu.mult)
        # ndst = 0.9 * rsqrt(deg[dst-slot])
        ndst_sb = const.tile([128, chunks], f32)
        nc.scalar.activation(ndst_sb[:], dcd_sb[:], Act.Sqrt)
        nc.vector.reciprocal(ndst_sb[:], ndst_sb[:])
        nc.vector.tensor_scalar(ndst_sb[:], ndst_sb[:], 1.0 - ALPHA, None, Alu.mult)

        call = 0
        for g in range(NG):
            buf = gpool.tile([128, GTILES * 128], bf16)
            for r in range(nsub):
                lo = r * sr
                hi = min(n_rows, (r + 1) * sr)
                nc.gpsimd.dma_gather(
                    out_ap=buf[:, r * NIC:(r + 1) * NIC]
                    .rearrange("p (t d) -> p t d", t=G * TR),
                    in_ap=feats[lo:hi, :],
                    idxs_ap=idx_sb[:, call * ICC:(call + 1) * ICC],
                    num_idxs=NIC,
                    num_idxs_reg=NIC,
                    elem_size=D,
                    single_packet=False,
                    queue_num=call % 4,
                )
                call += 1
            for cl in range(G):
                c = g * G + cl
                psum = ps_agg.tile([128, 128], f32, space="PSUM")
                for k in range(TT):
                    col = c * TT + k
                    r, t = divmod(k, TR)
                    off = ((r * G + cl) * TR + t) * 128
                    oh = ohpool.tile([128, 128], bf16)
                    eng = _engine_of(k, TT)
                    if eng == "dve":
                        nc.vector.tensor_scalar(
                            oh[:], iota_sb[:],
                            rel_sb[:, col:col + 1], nsrcf_sb[:, col:col + 1],
                            Alu.is_equal, Alu.mult,
                        )
                    elif eng == "gp":
                        nc.gpsimd.tensor_scalar(
                            oh[:], iota_sb[:],
                            rel_sb[:, col:col + 1], nsrcf_sb[:, col:col + 1],
                            Alu.is_equal, Alu.mult,
                        )
                    else:
                        tmp = tmppool.tile([128, 128], bf16)
                        nc.scalar.activation(
                            tmp[:], iota_sb[:], Act.Abs,
                            bias=nrel_sb[:, col:col + 1],
                        )
                        nc.scalar.activation(
                            oh[:], tmp[:], Act.Relu,
                            bias=nsrcf_sb[:, col:col + 1],
                            scale=negns_sb[:, col:col + 1],
                        )
                    nc.tensor.matmul(
                        psum[:],
                        lhsT=oh[:],
                        rhs=buf[:, off:off + 128],
                        start=(k == 0),
                        stop=(k == TT - 1),
                    )
                itile = ipool.tile([128, 128], f32)
                nc.sync.dma_start(out=itile[:], in_=initp[c * 128:(c + 1) * 128, :])
                h2 = epool.tile([128, 128], f32, tag="h2")
                nc.scalar.activation(h2[:], psum[:], Act.Copy,
                                     scale=ndst_sb[:, c:c + 1])
                # (0.9*ndst*agg + 0.1*init).T accumulated in PSUM
                ptr = ps_tr.tile([128, 128], f32, space="PSUM")
                nc.tensor.matmul(ptr[:], lhsT=h2[:], rhs=id_sb[:],
                                 start=True, stop=False)
                nc.tensor.matmul(ptr[:], lhsT=itile[:], rhs=id01_sb[:],
                                 start=False, stop=True)
                h3t = epool.tile([128, 128], f32, tag="h3t")
                nc.scalar.activation(h3t[:], ptr[:], Act.Copy)
                # h3 @ W.T + h3 accumulated in PSUM
                pmm = ps_mm.tile([128, 128], f32, space="PSUM")
                nc.tensor.matmul(
                    pmm[:], lhsT=h3t[:], rhs=wt_sb[:], start=True, stop=False
                )
                nc.tensor.matmul(
                    pmm[:], lhsT=h3t[:], rhs=id_sb[:], start=False, stop=True
                )
                ob = opool.tile([128, 128], f32)
                nc.scalar.activation(ob[:], pmm[:], Act.Relu, scale=BETA)
                nc.sync.dma_start(out=out[c * 128:(c + 1) * 128, :], in_=ob[:])

    nc.compile()
    _BUILD_CACHE[key] = nc
    return nc


def _install_ntff_shim():
    """antenv.axon_hooks is absent in this image; shim it and wire the real
    NTFF profiling hook via ctypes so trace=True works under axon."""
    import contextlib
    import ctypes
    import types

    try:
        from antenv import axon_hooks  # noqa: F401
        return
    except ImportError:
        pass
    import antenv

    mod = types.ModuleType("antenv.axon_hooks")
    _hook = [None]
    mod.set_axon_ntff_profile_hook = lambda h: _hook.__setitem__(0, h)
    mod.get_axon_ntff_profile_hook = lambda: _hook[0]
    sys.modules["antenv.axon_hooks"] = mod
    antenv.axon_hooks = mod
    try:
        lib = ctypes.CDLL("/opt/axon/libaxon_pjrt.so")
    except OSError:
        return
    if not hasattr(lib, "axon_start_nrt_profile"):
        return
    lib.axon_start_nrt_profile.argtypes = [
        ctypes.POINTER(ctypes.c_int64),
        ctypes.c_size_t,
    ]
    lib.axon_start_nrt_profile.restype = ctypes.c_int64
    lib.axon_stop_nrt_profile.argtypes = [ctypes.c_char_p]
    lib.axon_stop_nrt_profile.restype = ctypes.c_int64

    @contextlib.contextmanager
    def _hook_cm(output_dir, device_ids):
        import jax

        jax.devices()
        if device_ids:
            ids = (ctypes.c_int64 * len(device_ids))(*device_ids)
            rc = lib.axon_start_nrt_profile(ids, len(device_ids))
        else:
            rc = lib.axon_start_nrt_profile(None, 0)
        if rc != 0:
            raise RuntimeError(f"axon_start_nrt_profile rc={rc}")
        try:
            yield
        finally:
            rc = lib.axon_stop_nrt_profile(output_dir.encode())
            if rc != 0:
                print(f"WARNING: axon_stop_nrt_profile rc={rc}", flush=True)

    mod.set_axon_ntff_profile_hook(_hook_cm)


def _run(inputs, trace=False, trace_cores=None):
    from concourse import bass_utils

    if trace:
        _install_ntff_shim()
    features = np.ascontiguousarray(np.asarray(inputs["features"], dtype=F32))
    initial_features = np.ascontiguousarray(
        np.asarray(inputs["initial_features"], dtype=F32)
    )
    W = np.asarray(inputs["W"], dtype=F32)
    src = np.asarray(inputs["src"])
    dst = np.asarray(inputs["dst"])
    per_core, TR, CHUNKS, G = _host_prep(features, initial_features, W, src, dst)
    nc = _build(TR, CHUNKS, G)
    feats_bf = np.ascontiguousarray(features.astype(BF16))
    wt_np = np.ascontiguousarray(W.T)
    iota_np = np.ascontiguousarray(
        np.tile(np.arange(128, dtype=F32), (128, 1)).astype(BF16)
    )
    ident_np = np.eye(128, dtype=F32)
    ident01_np = np.eye(128, dtype=F32) * ALPHA
    in_maps = []
    for c in range(NC):
        pc = per_core[c]
        in_maps.append(
            dict(
                feats=feats_bf,
                wt=wt_np,
                iota=iota_np,
                ident=ident_np,
                ident01=ident01_np,
                eidx=pc["eidx"],
                erel=pc["erel"],
                negrel=pc["negrel"],
                edsg=pc["edsg"],
                dcd=pc["dcd"],
                initp=pc["initp"],
            )
        )
    res = bass_utils.run_bass_kernel_spmd(
        nc,
        in_maps,
        core_ids=list(range(NC)),
        trace=trace,
        trace_cores=trace_cores,
    )
    result = np.empty((N, D), F32)
    for c in range(NC):
        glob = per_core[c]["glob"].reshape(-1)
        oc = res.results[c]["out"]
        m = glob >= 0
        result[glob[m]] = oc[m]
    return result, res


def kernel(**inputs):
    return _run(inputs, trace=False)[0]


# revision 5
# speedup vs baseline: 4.1512x; 3.3977x over previous
"""GCNII layer on 8 TRN2 NeuronCores (Bass/Tile).

Strategy: partition nodes (and their incoming edges, bucketed by dst) across
the 8 cores; replicate the feature table (bf16) in every core's DRAM.  Per
core, nodes are greedily packed into chunks of 128 output slots balancing the
per-(chunk, src-subrange) edge counts so each (chunk,sub) fits TR 128-edge
tiles.  The feature table is split into 4 sub-tables of <32768 rows so
dma_gather's int16 indices reach every row; gathers are batched one call per
(4-chunk group, subrange) — the 4 calls of a group run on the 4 SWDGE queues
concurrently to parallelize Q7 descriptor generation.  Per 128-edge tile a
one-hot selection matrix (iota == slot) scaled by the fully-folded edge
weight 0.9*rsqrt(deg[src])*rsqrt(deg[dst]) is built on the Vector engine in
one dual-scalar op, and TensorE matmuls accumulate the edge-normalized
neighbor sum in fp32 PSUM — the PSUM result is already the blended
pre-residual value, so the epilogue is three wide [128,512] copies per
4-chunk block: PSUM copy, a transpose via PE (folding the 0.1*I initial
residual), and the identity-mapped W matmul with fused ReLU, written out
transposed ([D, SLOTS]) and un-permuted on the host.  Host-side work is
integer bucketing/layout only; all float math runs on device.
"""

import sys

if "/opt/trn_rl_repo" not in sys.path:
    sys.path.insert(0, "/opt/trn_rl_repo")

from contextlib import ExitStack

import ml_dtypes
import numpy as np

N, E, D, NC = 100000, 1600000, 128, 8
NPC = N // NC            # nodes per core: 12500
ALPHA, BETA = 0.1, 0.5
NSUB = 4                 # feature-table subranges (int16 index limit)
SR = 25000               # rows per subrange
BLK = 4                  # chunks per wide epilogue block (psum 512 free dim)

F32 = np.float32
BF16 = ml_dtypes.bfloat16


def _wrap_idx(seq):
    """dma_gather index layout: i -> [i % 16, i // 16], replicated to 128
    partitions (one copy per Q7 core)."""
    blk = seq.reshape(-1, 16).T
    return np.tile(blk, (8, 1))


def _balance_nodes(deg_sub, chunks):
    """Greedy 4-dim balancing: assign nodes (rows of deg_sub [n,4]) to
    `chunks` bins (<=128 nodes each) minimizing max per-(bin,sub) load."""
    n = deg_sub.shape[0]
    order = np.argsort(-deg_sub.sum(1), kind="stable")
    loads = np.zeros((chunks, NSUB), np.int64)
    counts = np.zeros(chunks, np.int64)
    chunk_of = np.empty(n, np.int64)
    slot_of = np.empty(n, np.int64)
    for i in order:
        score = np.max(loads + deg_sub[i], axis=1)
        score[counts >= 128] = 1 << 60
        c = int(np.argmin(score))
        chunk_of[i] = c
        slot_of[i] = counts[c]
        counts[c] += 1
        loads[c] += deg_sub[i]
    return chunk_of, slot_of, loads


def _host_prep(features, initial_features, W, src, dst):
    """Integer-only bucketing/layout prep -> per-core device arrays."""
    src = np.ascontiguousarray(src).astype(np.int64, copy=False)
    dst = np.ascontiguousarray(dst).astype(np.int64, copy=False)
    deg = np.bincount(dst, minlength=N)
    degc = np.maximum(deg, 1).astype(np.int64)
    core_of = dst // NPC

    CHUNKS = 104
    cores_tmp = []
    max_load = 0
    for c in range(NC):
        em = core_of == c
        e_src = src[em]
        e_loc = dst[em] - c * NPC
        e_sub = e_src // SR
        deg_sub = np.zeros((NPC, NSUB), np.int64)
        np.add.at(deg_sub, (e_loc, e_sub), 1)
        chunk_of, slot_of, loads = _balance_nodes(deg_sub, CHUNKS)
        max_load = max(max_load, int(loads.max()))
        cores_tmp.append((e_src, e_loc, e_sub, chunk_of, slot_of))
    TR = (max_load + 127) // 128         # tiles per (chunk, subrange)
    TT = NSUB * TR                       # tiles per chunk
    cap = TR * 128
    SLOTS = CHUNKS * 128
    G = 4                                # chunks per gather group
    per_core = []
    for c in range(NC):
        e_src, e_loc, e_sub, chunk_of, slot_of = cores_tmp[c]
        e_chunk = chunk_of[e_loc]
        e_slot = slot_of[e_loc]
        o = np.lexsort((e_src, e_sub, e_chunk))
        e_src, e_slot, e_chunk, e_sub = e_src[o], e_slot[o], e_chunk[o], e_sub[o]
        e_ddeg = degc[e_loc[o] + c * NPC]    # deg[dst] per (sorted) edge
        e_sdeg = degc[e_src]
        cnt = np.bincount(e_chunk * NSUB + e_sub, minlength=CHUNKS * NSUB)
        # [CHUNKS, NSUB, cap] per-(chunk,subrange) padded segments
        idx_arr = np.zeros((CHUNKS, NSUB, cap), np.int16)
        rel_arr = np.full((CHUNKS, NSUB, cap), -1.0, F32)
        dsd_arr = np.ones((CHUNKS, NSUB, cap), F32)   # deg[src]*deg[dst]
        starts = np.zeros(CHUNKS * NSUB, np.int64)
        starts[1:] = np.cumsum(cnt)[:-1]
        pos = np.arange(len(e_src)) - starts[e_chunk * NSUB + e_sub]
        idx_arr[e_chunk, e_sub, pos] = (e_src - e_sub * SR).astype(np.int16)
        rel_arr[e_chunk, e_sub, pos] = e_slot
        dsd_arr[e_chunk, e_sub, pos] = (e_sdeg * e_ddeg).astype(F32)
        # gather-call order: (group g, sub r, chunk-local, tile t, part p)
        NG = CHUNKS // G
        idx_g = idx_arr.reshape(NG, G, NSUB, cap)
        idx_flat = idx_g.transpose(0, 2, 1, 3).reshape(-1)
        idx_dev = _wrap_idx(idx_flat).astype(np.int16)   # [128, COLS*8]

        def dev(a):   # [CHUNKS, NSUB, cap] -> [128, COLS] table, col=c*TT+k
            return np.ascontiguousarray(a.reshape(CHUNKS * TT, 128).T)

        # node layout tables
        nodelist = np.full((CHUNKS, 128), -1, np.int64)
        nodelist[chunk_of, slot_of] = np.arange(NPC)
        glob = np.where(nodelist >= 0, nodelist + c * NPC, -1)
        init_perm = np.zeros((SLOTS, D), F32)
        gv = glob.reshape(-1)
        init_perm[gv >= 0] = initial_features[gv[gv >= 0]]
        per_core.append(
            dict(
                eidx=np.ascontiguousarray(idx_dev),
                erel=np.ascontiguousarray(dev(rel_arr)),
                edsd=dev(dsd_arr),
                initp=init_perm,
                glob=glob,
            )
        )
    return per_core, TR, CHUNKS, G


_BUILD_CACHE = {}


def _build(TR, chunks, G, n_rows=N, nsub=NSUB, sr=SR):
    key = (TR, chunks, G, n_rows, nsub, sr)
    if key in _BUILD_CACHE:
        return _BUILD_CACHE[key]
    import concourse.bacc as bacc
    import concourse.bass as bass  # noqa: F401
    import concourse.mybir as mybir
    import concourse.tile as tile

    f32 = mybir.dt.float32
    bf16 = mybir.dt.bfloat16
    i16 = mybir.dt.int16
    Alu = mybir.AluOpType
    Act = mybir.ActivationFunctionType

    TT = nsub * TR
    SLOTS_ = chunks * 128
    COLS = chunks * TT               # total edge-tile columns
    IDXC = COLS * 8                  # idx cols (int16, 16-wrap => /16*128)
    NG = chunks // G                 # gather groups
    GTILES = G * TT                  # tiles per group
    NIC = G * TR * 128               # idxs per gather call
    ICC = NIC // 16                  # idx cols per gather call
    NB = chunks // BLK               # wide epilogue blocks

    nc = bacc.Bacc("TRN2", target_bir_lowering=False, num_swdge_queues=4)
    feats = nc.dram_tensor("feats", [n_rows, D], bf16, kind="ExternalInput")
    wt = nc.dram_tensor("wt", [D, D], f32, kind="ExternalInput")
    iota = nc.dram_tensor("iota", [128, 128], bf16, kind="ExternalInput")
    ident = nc.dram_tensor("ident", [128, 128], f32, kind="ExternalInput")
    ident01 = nc.dram_tensor("ident01", [128, 128], f32, kind="ExternalInput")
    eidx = nc.dram_tensor("eidx", [128, IDXC], i16, kind="ExternalInput")
    erel = nc.dram_tensor("erel", [128, COLS], f32, kind="ExternalInput")
    edsd = nc.dram_tensor("edsd", [128, COLS], f32, kind="ExternalInput")
    initp = nc.dram_tensor("initp", [SLOTS_, D], f32, kind="ExternalInput")
    outT = nc.dram_tensor("outT", [D, SLOTS_], f32, kind="ExternalOutput")

    with tile.TileContext(nc) as tc, ExitStack() as ctx:
        const = ctx.enter_context(tc.tile_pool(name="const", bufs=1))
        gpool = ctx.enter_context(tc.tile_pool(name="g", bufs=3))
        ohpool = ctx.enter_context(tc.tile_pool(name="oh", bufs=64))
        epool = ctx.enter_context(tc.tile_pool(name="ep", bufs=3))
        ipool = ctx.enter_context(tc.tile_pool(name="init", bufs=6))
        opool = ctx.enter_context(tc.tile_pool(name="ob", bufs=3))
        ps_agg = ctx.enter_context(tc.tile_pool(name="psagg", bufs=3, space="PSUM"))
        ps_tr = ctx.enter_context(tc.tile_pool(name="pstr", bufs=2, space="PSUM"))
        ps_mm = ctx.enter_context(tc.tile_pool(name="psmm", bufs=2, space="PSUM"))

        iota_sb = const.tile([128, 128], bf16)
        nc.sync.dma_start(out=iota_sb[:], in_=iota[:])
        wt_sb = const.tile([128, 128], f32)
        nc.sync.dma_start(out=wt_sb[:], in_=wt[:])
        id_sb = const.tile([128, 128], f32)
        nc.sync.dma_start(out=id_sb[:], in_=ident[:])
        id01_sb = const.tile([128, 128], f32)
        nc.sync.dma_start(out=id01_sb[:], in_=ident01[:])
        idx_sb = const.tile([128, IDXC], i16)
        nc.sync.dma_start(out=idx_sb[:], in_=eidx[:])
        rel_sb = const.tile([128, COLS], f32)
        nc.sync.dma_start(out=rel_sb[:], in_=erel[:])
        dsd_sb = const.tile([128, COLS], f32)
        nc.sync.dma_start(out=dsd_sb[:], in_=edsd[:])

        # scl = 0.9 * rsqrt(deg[src]*deg[dst]) per edge (fully folded weight)
        scl_sb = const.tile([128, COLS], f32)
        nc.scalar.activation(scl_sb[:], dsd_sb[:], Act.Sqrt)
        nc.vector.reciprocal(scl_sb[:], scl_sb[:])
        nc.vector.tensor_scalar(scl_sb[:], scl_sb[:], 1.0 - ALPHA, None, Alu.mult)

        call = 0
        for g in range(NG):
            buf = gpool.tile([128, GTILES * 128], bf16)
            for r in range(nsub):
                lo = r * sr
                hi = min(n_rows, (r + 1) * sr)
                nc.gpsimd.dma_gather(
                    out_ap=buf[:, r * NIC:(r + 1) * NIC]
                    .rearrange("p (t d) -> p t d", t=G * TR),
                    in_ap=feats[lo:hi, :],
                    idxs_ap=idx_sb[:, call * ICC:(call + 1) * ICC],
                    num_idxs=NIC,
                    num_idxs_reg=NIC,
                    elem_size=D,
                    single_packet=False,
                    queue_num=call % 4,
                )
                call += 1
            for cl in range(G):
                c = g * G + cl
                bi, bc = divmod(c, BLK)
                if bc == 0:
                    psw = ps_agg.tile([128, BLK * 128], f32, space="PSUM",
                                      name="psw")
                for k in range(TT):
                    col = c * TT + k
                    r, t = divmod(k, TR)
                    off = ((r * G + cl) * TR + t) * 128
                    oh = ohpool.tile([128, 128], bf16)
                    nc.vector.tensor_scalar(
                        oh[:], iota_sb[:],
                        rel_sb[:, col:col + 1], scl_sb[:, col:col + 1],
                        Alu.is_equal, Alu.mult,
                    )
                    nc.tensor.matmul(
                        psw[:, bc * 128:(bc + 1) * 128],
                        lhsT=oh[:],
                        rhs=buf[:, off:off + 128],
                        start=(k == 0),
                        stop=(k == TT - 1),
                    )
                if bc == BLK - 1:
                    # wide epilogue for chunks [bi*BLK, (bi+1)*BLK)
                    h2w = epool.tile([128, BLK * 128], f32, tag="h2w")
                    nc.scalar.activation(h2w[:], psw[:], Act.Copy)
                    ptrw = ps_tr.tile([128, BLK * 128], f32, space="PSUM",
                                      name="ptrw")
                    for j in range(BLK):
                        cj = bi * BLK + j
                        itile = ipool.tile([128, 128], f32, name="itile")
                        nc.sync.dma_start(
                            out=itile[:],
                            in_=initp[cj * 128:(cj + 1) * 128, :])
                        nc.tensor.matmul(
                            ptrw[:, j * 128:(j + 1) * 128],
                            lhsT=h2w[:, j * 128:(j + 1) * 128], rhs=id_sb[:],
                            start=True, stop=False)
                        nc.tensor.matmul(
                            ptrw[:, j * 128:(j + 1) * 128],
                            lhsT=itile[:], rhs=id01_sb[:],
                            start=False, stop=True)
                    h3tw = epool.tile([128, BLK * 128], f32, tag="h3tw")
                    nc.scalar.activation(h3tw[:], ptrw[:], Act.Copy)
                    pmmw = ps_mm.tile([128, BLK * 128], f32, space="PSUM",
                                      name="pmmw")
                    nc.tensor.matmul(pmmw[:], lhsT=wt_sb[:], rhs=h3tw[:],
                                     start=True, stop=False)
                    nc.tensor.matmul(pmmw[:], lhsT=id_sb[:], rhs=h3tw[:],
                                     start=False, stop=True)
                    obw = opool.tile([128, BLK * 128], f32)
                    nc.scalar.activation(obw[:], pmmw[:], Act.Relu, scale=BETA)
                    nc.sync.dma_start(
                        out=outT[:, bi * BLK * 128:(bi + 1) * BLK * 128],
                        in_=obw[:])

    nc.compile()
    _BUILD_CACHE[key] = nc
    return nc


def _install_ntff_shim():
    """antenv.axon_hooks is absent in this image; shim it and wire the real
    NTFF profiling hook via ctypes so trace=True works under axon."""
    import contextlib
    import ctypes
    import types

    try:
        from antenv import axon_hooks  # noqa: F401
        return
    except ImportError:
        pass
    import antenv

    mod = types.ModuleType("antenv.axon_hooks")
    _hook = [None]
    mod.set_axon_ntff_profile_hook = lambda h: _hook.__setitem__(0, h)
    mod.get_axon_ntff_profile_hook = lambda: _hook[0]
    sys.modules["antenv.axon_hooks"] = mod
    antenv.axon_hooks = mod
    try:
        lib = ctypes.CDLL("/opt/axon/libaxon_pjrt.so")
    except OSError:
        return
    if not hasattr(lib, "axon_start_nrt_profile"):
        return
    lib.axon_start_nrt_profile.argtypes = [
        ctypes.POINTER(ctypes.c_int64),
        ctypes.c_size_t,
    ]
    lib.axon_start_nrt_profile.restype = ctypes.c_int64
    lib.axon_stop_nrt_profile.argtypes = [ctypes.c_char_p]
    lib.axon_stop_nrt_profile.restype = ctypes.c_int64

    @contextlib.contextmanager
    def _hook_cm(output_dir, device_ids):
        import jax

        jax.devices()
        if device_ids:
            ids = (ctypes.c_int64 * len(device_ids))(*device_ids)
            rc = lib.axon_start_nrt_profile(ids, len(device_ids))
        else:
            rc = lib.axon_start_nrt_profile(None, 0)
        if rc != 0:
            raise RuntimeError(f"axon_start_nrt_profile rc={rc}")
        try:
            yield
        finally:
            rc = lib.axon_stop_nrt_profile(output_dir.encode())
            if rc != 0:
                print(f"WARNING: axon_stop_nrt_profile rc={rc}", flush=True)

    mod.set_axon_ntff_profile_hook(_hook_cm)


def _run(inputs, trace=False, trace_cores=None):
    from concourse import bass_utils

    if trace:
        _install_ntff_shim()
    features = np.ascontiguousarray(np.asarray(inputs["features"], dtype=F32))
    initial_features = np.ascontiguousarray(
        np.asarray(inputs["initial_features"], dtype=F32)
    )
    W = np.asarray(inputs["W"], dtype=F32)
    src = np.asarray(inputs["src"])
    dst = np.asarray(inputs["dst"])
    per_core, TR, CHUNKS, G = _host_prep(features, initial_features, W, src, dst)
    nc = _build(TR, CHUNKS, G)
    feats_bf = np.ascontiguousarray(features.astype(BF16))
    wt_np = np.ascontiguousarray(W.T)
    iota_np = np.ascontiguousarray(
        np.tile(np.arange(128, dtype=F32), (128, 1)).astype(BF16)
    )
    ident_np = np.eye(128, dtype=F32)
    ident01_np = np.eye(128, dtype=F32) * ALPHA
    in_maps = []
    for c in range(NC):
        pc = per_core[c]
        in_maps.append(
            dict(
                feats=feats_bf,
                wt=wt_np,
                iota=iota_np,
                ident=ident_np,
                ident01=ident01_np,
                eidx=pc["eidx"],
                erel=pc["erel"],
                edsd=pc["edsd"],
                initp=pc["initp"],
            )
        )
    res = bass_utils.run_bass_kernel_spmd(
        nc,
        in_maps,
        core_ids=list(range(NC)),
        trace=trace,
        trace_cores=trace_cores,
    )
    result = np.empty((N, D), F32)
    for c in range(NC):
        glob = per_core[c]["glob"].reshape(-1)
        oc = np.ascontiguousarray(res.results[c]["outT"].T)
        m = glob >= 0
        result[glob[m]] = oc[m]
    return result, res


def kernel(**inputs):
    return _run(inputs, trace=False)[0]


# revision 7
# speedup vs baseline: 7.2607x; 1.7491x over previous
"""GCNII layer on 8 TRN2 NeuronCores (Bass/Tile).

Strategy: partition nodes (and their incoming edges, bucketed by dst) across
the 8 cores; replicate the feature table (bf16) in every core's DRAM.  Per
core, nodes are greedily packed into chunks of 128 output slots balancing the
per-(chunk, src-subrange) edge counts so each (chunk,sub) fits TR 128-edge
tiles.  The feature table is split into 4 sub-tables of <32768 rows so
dma_gather's int16 indices reach every row; gathers are batched one call per
(4-chunk group, subrange) — the 4 calls of a group run on the 4 SWDGE queues
concurrently to parallelize Q7 descriptor generation.  Per 128-edge tile a
one-hot selection matrix (iota == slot) scaled by the fully-folded edge
weight 0.9*rsqrt(deg[src])*rsqrt(deg[dst]) is built on the Vector engine in
one dual-scalar op, and TensorE matmuls accumulate the edge-normalized
neighbor sum in fp32 PSUM — the PSUM result is already the blended
pre-residual value, so the epilogue is three wide [128,512] copies per
4-chunk block: PSUM copy, a transpose via PE (folding the 0.1*I initial
residual), and the identity-mapped W matmul with fused ReLU, written out
transposed ([D, SLOTS]) and un-permuted on the host.  Host-side work is
integer bucketing/layout only; all float math runs on device.
"""

import sys

if "/opt/trn_rl_repo" not in sys.path:
    sys.path.insert(0, "/opt/trn_rl_repo")

from contextlib import ExitStack

import ml_dtypes
import numpy as np

N, E, D, NC = 100000, 1600000, 128, 8
NPC = N // NC            # nodes per core: 12500
ALPHA, BETA = 0.1, 0.5
NSUB = 4                 # feature-table subranges (int16 index limit)
SR = 25000               # rows per subrange
BLK = 4                  # chunks per wide epilogue block (psum 512 free dim)

F32 = np.float32
BF16 = ml_dtypes.bfloat16


def _wrap_idx(seq):
    """dma_gather index layout: i -> [i % 16, i // 16], replicated to 128
    partitions (one copy per Q7 core)."""
    blk = seq.reshape(-1, 16).T
    return np.tile(blk, (8, 1))


def _balance_nodes(deg_sub, chunks):
    """Greedy 4-dim balancing: assign nodes (rows of deg_sub [n,4]) to
    `chunks` bins (<=128 nodes each) minimizing max per-(bin,sub) load."""
    n = deg_sub.shape[0]
    order = np.argsort(-deg_sub.sum(1), kind="stable")
    loads = np.zeros((chunks, NSUB), np.int64)
    counts = np.zeros(chunks, np.int64)
    chunk_of = np.empty(n, np.int64)
    slot_of = np.empty(n, np.int64)
    for i in order:
        score = np.max(loads + deg_sub[i], axis=1)
        score[counts >= 128] = 1 << 60
        c = int(np.argmin(score))
        chunk_of[i] = c
        slot_of[i] = counts[c]
        counts[c] += 1
        loads[c] += deg_sub[i]
    return chunk_of, slot_of, loads


def _host_prep(features, initial_features, W, src, dst):
    """Integer-only bucketing/layout prep -> per-core device arrays."""
    src = np.ascontiguousarray(src).astype(np.int64, copy=False)
    dst = np.ascontiguousarray(dst).astype(np.int64, copy=False)
    deg = np.bincount(dst, minlength=N)
    degc = np.maximum(deg, 1).astype(np.int64)
    core_of = dst // NPC

    CHUNKS = 104
    cores_tmp = []
    max_load = 0
    for c in range(NC):
        em = core_of == c
        e_src = src[em]
        e_loc = dst[em] - c * NPC
        e_sub = e_src // SR
        deg_sub = np.zeros((NPC, NSUB), np.int64)
        np.add.at(deg_sub, (e_loc, e_sub), 1)
        chunk_of, slot_of, loads = _balance_nodes(deg_sub, CHUNKS)
        max_load = max(max_load, int(loads.max()))
        cores_tmp.append((e_src, e_loc, e_sub, chunk_of, slot_of))
    TR = (max_load + 127) // 128         # tiles per (chunk, subrange)
    TT = NSUB * TR                       # tiles per chunk
    cap = TR * 128
    SLOTS = CHUNKS * 128
    G = 4                                # chunks per gather group
    per_core = []
    for c in range(NC):
        e_src, e_loc, e_sub, chunk_of, slot_of = cores_tmp[c]
        e_chunk = chunk_of[e_loc]
        e_slot = slot_of[e_loc]
        o = np.lexsort((e_src, e_sub, e_chunk))
        e_src, e_slot, e_chunk, e_sub = e_src[o], e_slot[o], e_chunk[o], e_sub[o]
        e_ddeg = degc[e_loc[o] + c * NPC]    # deg[dst] per (sorted) edge
        e_sdeg = degc[e_src]
        cnt = np.bincount(e_chunk * NSUB + e_sub, minlength=CHUNKS * NSUB)
        # [CHUNKS, NSUB, cap] per-(chunk,subrange) padded segments
        idx_arr = np.zeros((CHUNKS, NSUB, cap), np.int16)
        rel_arr = np.full((CHUNKS, NSUB, cap), -1.0, F32)
        dsd_arr = np.ones((CHUNKS, NSUB, cap), F32)   # deg[src]*deg[dst]
        starts = np.zeros(CHUNKS * NSUB, np.int64)
        starts[1:] = np.cumsum(cnt)[:-1]
        pos = np.arange(len(e_src)) - starts[e_chunk * NSUB + e_sub]
        idx_arr[e_chunk, e_sub, pos] = (e_src - e_sub * SR).astype(np.int16)
        rel_arr[e_chunk, e_sub, pos] = e_slot
        dsd_arr[e_chunk, e_sub, pos] = (e_sdeg * e_ddeg).astype(F32)
        # gather-call order: (group g, sub r, chunk-local, tile t, part p)
        NG = CHUNKS // G
        idx_g = idx_arr.reshape(NG, G, NSUB, cap)
        idx_flat = idx_g.transpose(0, 2, 1, 3).reshape(-1)
        idx_dev = _wrap_idx(idx_flat).astype(np.int16)   # [128, COLS*8]

        def dev(a):   # [CHUNKS, NSUB, cap] -> [128, COLS] table, col=c*TT+k
            return np.ascontiguousarray(a.reshape(CHUNKS * TT, 128).T)

        # node layout tables
        nodelist = np.full((CHUNKS, 128), -1, np.int64)
        nodelist[chunk_of, slot_of] = np.arange(NPC)
        glob = np.where(nodelist >= 0, nodelist + c * NPC, -1)
        init_perm = np.zeros((SLOTS, D), F32)
        gv = glob.reshape(-1)
        init_perm[gv >= 0] = initial_features[gv[gv >= 0]]
        per_core.append(
            dict(
                eidx=np.ascontiguousarray(idx_dev),
                erel=np.ascontiguousarray(dev(rel_arr).astype(BF16)),
                edsd=dev(dsd_arr),
                initp=init_perm,
                glob=glob,
            )
        )
    return per_core, TR, CHUNKS, G


_BUILD_CACHE = {}


def _build(TR, chunks, G, n_rows=N, nsub=NSUB, sr=SR):
    key = (TR, chunks, G, n_rows, nsub, sr)
    if key in _BUILD_CACHE:
        return _BUILD_CACHE[key]
    import concourse.bacc as bacc
    import concourse.bass as bass  # noqa: F401
    import concourse.mybir as mybir
    import concourse.tile as tile

    f32 = mybir.dt.float32
    bf16 = mybir.dt.bfloat16
    i16 = mybir.dt.int16
    Alu = mybir.AluOpType
    Act = mybir.ActivationFunctionType

    TT = nsub * TR
    SLOTS_ = chunks * 128
    COLS = chunks * TT               # total edge-tile columns
    IDXC = COLS * 8                  # idx cols (int16, 16-wrap => /16*128)
    NG = chunks // G                 # gather groups
    GTILES = G * TT                  # tiles per group
    NIC = G * TR * 128               # idxs per gather call
    ICC = NIC // 16                  # idx cols per gather call
    NB = chunks // BLK               # wide epilogue blocks

    nc = bacc.Bacc("TRN2", target_bir_lowering=False, num_swdge_queues=4)
    feats = nc.dram_tensor("feats", [n_rows, D], bf16, kind="ExternalInput")
    wt = nc.dram_tensor("wt", [D, D], f32, kind="ExternalInput")
    iota = nc.dram_tensor("iota", [128, 128], bf16, kind="ExternalInput")
    ident = nc.dram_tensor("ident", [128, 128], f32, kind="ExternalInput")
    ident01 = nc.dram_tensor("ident01", [128, 128], f32, kind="ExternalInput")
    eidx = nc.dram_tensor("eidx", [128, IDXC], i16, kind="ExternalInput")
    erel = nc.dram_tensor("erel", [128, COLS], bf16, kind="ExternalInput")
    edsd = nc.dram_tensor("edsd", [128, COLS], f32, kind="ExternalInput")
    initp = nc.dram_tensor("initp", [SLOTS_, D], f32, kind="ExternalInput")
    outT = nc.dram_tensor("outT", [D, SLOTS_], f32, kind="ExternalOutput")

    with tile.TileContext(nc) as tc, ExitStack() as ctx:
        const = ctx.enter_context(tc.tile_pool(name="const", bufs=1))
        gpools = [ctx.enter_context(tc.tile_pool(name=f"g{r}", bufs=3))
                  for r in range(nsub)]
        oh1pool = ctx.enter_context(tc.tile_pool(name="oh1", bufs=3))
        oh2pool = ctx.enter_context(tc.tile_pool(name="oh2", bufs=4))
        epool = ctx.enter_context(tc.tile_pool(name="ep", bufs=4))
        ipool = ctx.enter_context(tc.tile_pool(name="init", bufs=6))
        opool = ctx.enter_context(tc.tile_pool(name="ob", bufs=3))
        ps_agg = ctx.enter_context(tc.tile_pool(name="psagg", bufs=2, space="PSUM"))
        ps_tr = ctx.enter_context(tc.tile_pool(name="pstr", bufs=2, space="PSUM"))
        ps_mm = ctx.enter_context(tc.tile_pool(name="psmm", bufs=2, space="PSUM"))

        iota_sb = const.tile([128, 128], bf16)
        nc.sync.dma_start(out=iota_sb[:], in_=iota[:])
        wt_sb = const.tile([128, 128], f32)
        nc.sync.dma_start(out=wt_sb[:], in_=wt[:])
        id_sb = const.tile([128, 128], f32)
        nc.sync.dma_start(out=id_sb[:], in_=ident[:])
        id01_sb = const.tile([128, 128], f32)
        nc.sync.dma_start(out=id01_sb[:], in_=ident01[:])
        idx_sb = const.tile([128, IDXC], i16)
        nc.sync.dma_start(out=idx_sb[:], in_=eidx[:])
        rel_sb = const.tile([128, COLS], bf16)
        nc.sync.dma_start(out=rel_sb[:], in_=erel[:])
        dsd_sb = const.tile([128, COLS], f32)
        nc.sync.dma_start(out=dsd_sb[:], in_=edsd[:])

        # scl = 0.9 * rsqrt(deg[src]*deg[dst]) per edge (fully folded weight)
        sclf_sb = const.tile([128, COLS], f32)
        nc.scalar.activation(sclf_sb[:], dsd_sb[:], Act.Sqrt)
        nc.vector.reciprocal(sclf_sb[:], sclf_sb[:])
        nc.vector.tensor_scalar(sclf_sb[:], sclf_sb[:], 1.0 - ALPHA, None,
                                Alu.mult)
        scl_sb = const.tile([128, COLS], bf16)
        nc.scalar.activation(scl_sb[:], sclf_sb[:], Act.Copy)

        call = 0
        for g in range(NG):
            bufs = []
            for r in range(nsub):
                lo = r * sr
                hi = min(n_rows, (r + 1) * sr)
                bufr = gpools[r].tile([128, NIC], bf16, name=f"b{r}")
                nc.gpsimd.dma_gather(
                    out_ap=bufr[:].rearrange("p (t d) -> p t d", t=G * TR),
                    in_ap=feats[lo:hi, :],
                    idxs_ap=idx_sb[:, call * ICC:(call + 1) * ICC],
                    num_idxs=NIC,
                    num_idxs_reg=NIC,
                    elem_size=D,
                    single_packet=False,
                    queue_num=call % 4,
                )
                bufs.append(bufr)
                call += 1
            for cl in range(G):
                c = g * G + cl
                bi, bc = divmod(c, BLK)
                if bc == 0:
                    pswA = ps_agg.tile([128, BLK * 128], f32, space="PSUM",
                                       name="pswA")
                    pswB = ps_agg.tile([128, BLK * 128], f32, space="PSUM",
                                       name="pswB")
                # wide one-hot build: is_eq then scale, one chunk at a time
                oh1 = oh1pool.tile([128, TT * 128], bf16, name="oh1")
                nc.vector.tensor_tensor(
                    oh1[:].rearrange("p (k s) -> p k s", k=TT),
                    iota_sb[:].unsqueeze(1).broadcast_to([128, TT, 128]),
                    rel_sb[:, c * TT:(c + 1) * TT]
                    .unsqueeze(-1).broadcast_to([128, TT, 128]),
                    Alu.is_equal)
                oh2 = oh2pool.tile([128, TT * 128], bf16, name="oh2")
                nc.vector.tensor_tensor(
                    oh2[:].rearrange("p (k s) -> p k s", k=TT),
                    oh1[:].rearrange("p (k s) -> p k s", k=TT),
                    scl_sb[:, c * TT:(c + 1) * TT]
                    .unsqueeze(-1).broadcast_to([128, TT, 128]),
                    Alu.mult)
                nhalf = TT // 2
                for k in range(TT):
                    r, t = divmod(k, TR)
                    off = (cl * TR + t) * 128
                    psw = pswA if k % 2 == 0 else pswB
                    nc.tensor.matmul(
                        psw[:, bc * 128:(bc + 1) * 128],
                        lhsT=oh2[:, k * 128:(k + 1) * 128],
                        rhs=bufs[r][:, off:off + 128],
                        start=(k < 2),
                        stop=(k >= TT - 2),
                    )
                if bc == BLK - 1:
                    # wide epilogue for chunks [bi*BLK, (bi+1)*BLK)
                    h2a = epool.tile([128, BLK * 128], f32, tag="h2a")
                    nc.scalar.activation(h2a[:], pswA[:], Act.Copy)
                    h2b = epool.tile([128, BLK * 128], f32, tag="h2b")
                    nc.scalar.activation(h2b[:], pswB[:], Act.Copy)
                    ptrw = ps_tr.tile([128, BLK * 128], f32, space="PSUM",
                                      name="ptrw")
                    for j in range(BLK):
                        cj = bi * BLK + j
                        itile = ipool.tile([128, 128], f32, name="itile")
                        nc.sync.dma_start(
                            out=itile[:],
                            in_=initp[cj * 128:(cj + 1) * 128, :])
                        nc.tensor.matmul(
                            ptrw[:, j * 128:(j + 1) * 128],
                            lhsT=h2a[:, j * 128:(j + 1) * 128], rhs=id_sb[:],
                            start=True, stop=False)
                        nc.tensor.matmul(
                            ptrw[:, j * 128:(j + 1) * 128],
                            lhsT=h2b[:, j * 128:(j + 1) * 128], rhs=id_sb[:],
                            start=False, stop=False)
                        nc.tensor.matmul(
                            ptrw[:, j * 128:(j + 1) * 128],
                            lhsT=itile[:], rhs=id01_sb[:],
                            start=False, stop=True)
                    h3tw = epool.tile([128, BLK * 128], f32, tag="h3tw")
                    nc.scalar.activation(h3tw[:], ptrw[:], Act.Copy)
                    pmmw = ps_mm.tile([128, BLK * 128], f32, space="PSUM",
                                      name="pmmw")
                    nc.tensor.matmul(pmmw[:], lhsT=wt_sb[:], rhs=h3tw[:],
                                     start=True, stop=False)
                    nc.tensor.matmul(pmmw[:], lhsT=id_sb[:], rhs=h3tw[:],
                                     start=False, stop=True)
                    obw = opool.tile([128, BLK * 128], f32)
                    nc.scalar.activation(obw[:], pmmw[:], Act.Relu, scale=BETA)
                    nc.sync.dma_start(
                        out=outT[:, bi * BLK * 128:(bi + 1) * BLK * 128],
                        in_=obw[:])

    nc.compile()
    _BUILD_CACHE[key] = nc
    return nc


def _install_ntff_shim():
    """antenv.axon_hooks is absent in this image; shim it and wire the real
    NTFF profiling hook via ctypes so trace=True works under axon."""
    import contextlib
    import ctypes
    import types

    try:
        from antenv import axon_hooks  # noqa: F401
        return
    except ImportError:
        pass
    import antenv

    mod = types.ModuleType("antenv.axon_hooks")
    _hook = [None]
    mod.set_axon_ntff_profile_hook = lambda h: _hook.__setitem__(0, h)
    mod.get_axon_ntff_profile_hook = lambda: _hook[0]
    sys.modules["antenv.axon_hooks"] = mod
    antenv.axon_hooks = mod
    try:
        lib = ctypes.CDLL("/opt/axon/libaxon_pjrt.so")
    except OSError:
        return
    if not hasattr(lib, "axon_start_nrt_profile"):
        return
    lib.axon_start_nrt_profile.argtypes = [
        ctypes.POINTER(ctypes.c_int64),
        ctypes.c_size_t,
    ]
    lib.axon_start_nrt_profile.restype = ctypes.c_int64
    lib.axon_stop_nrt_profile.argtypes = [ctypes.c_char_p]
    lib.axon_stop_nrt_profile.restype = ctypes.c_int64

    @contextlib.contextmanager
    def _hook_cm(output_dir, device_ids):
        import jax

        jax.devices()
        if device_ids:
            ids = (ctypes.c_int64 * len(device_ids))(*device_ids)
            rc = lib.axon_start_nrt_profile(ids, len(device_ids))
        else:
            rc = lib.axon_start_nrt_profile(None, 0)
        if rc != 0:
            raise RuntimeError(f"axon_start_nrt_profile rc={rc}")
        try:
            yield
        finally:
            rc = lib.axon_stop_nrt_profile(output_dir.encode())
            if rc != 0:
                print(f"WARNING: axon_stop_nrt_profile rc={rc}", flush=True)

    mod.set_axon_ntff_profile_hook(_hook_cm)


def _run(inputs, trace=False, trace_cores=None):
    from concourse import bass_utils

    if trace:
        _install_ntff_shim()
    features = np.ascontiguousarray(np.asarray(inputs["features"], dtype=F32))
    initial_features = np.ascontiguousarray(
        np.asarray(inputs["initial_features"], dtype=F32)
    )
    W = np.asarray(inputs["W"], dtype=F32)
    src = np.asarray(inputs["src"])
    dst = np.asarray(inputs["dst"])
    per_core, TR, CHUNKS, G = _host_prep(features, initial_features, W, src, dst)
    nc = _build(TR, CHUNKS, G)
    feats_bf = np.ascontiguousarray(features.astype(BF16))
    wt_np = np.ascontiguousarray(W.T)
    iota_np = np.ascontiguousarray(
        np.tile(np.arange(128, dtype=F32), (128, 1)).astype(BF16)
    )
    ident_np = np.eye(128, dtype=F32)
    ident01_np = np.eye(128, dtype=F32) * ALPHA
    in_maps = []
    for c in range(NC):
        pc = per_core[c]
        in_maps.append(
            dict(
                feats=feats_bf,
                wt=wt_np,
                iota=iota_np,
                ident=ident_np,
                ident01=ident01_np,
                eidx=pc["eidx"],
                erel=pc["erel"],
                edsd=pc["edsd"],
                initp=pc["initp"],
            )
        )
    res = bass_utils.run_bass_kernel_spmd(
        nc,
        in_maps,
        core_ids=list(range(NC)),
        trace=trace,
        trace_cores=trace_cores,
    )
    result = np.empty((N, D), F32)
    for c in range(NC):
        glob = per_core[c]["glob"].reshape(-1)
        oc = np.ascontiguousarray(res.results[c]["outT"].T)
        m = glob >= 0
        result[glob[m]] = oc[m]
    return result, res


def kernel(**inputs):
    return _run(inputs, trace=False)[0]
